# revision 15
# baseline (speedup 1.0000x reference)
"""Sliding-window attention (WINDOW=129) Trainium2 Bass kernel, v2.

Problem: x[B=2, N=2048, C=768] -> qkv proj -> 12-head sliding-window
attention (half-window 64) -> output proj + bias.

Sharding: sequence-parallel over 8 cores: core c handles batch b = c//4,
query chunk s = c%4 (512 queries), with a 64-row halo each side for K/V.
Weights replicated. No collectives.

v2 changes vs v1:
  - q/k kept PAIR-STACKED in SBUF ([128, pair, n]; head 2g at rows 0-63,
    2g+1 at rows 64-127): psum evacuation is ONE full-height copy per
    chunk (split DVE/ACT) instead of copy+stream_shuffle+copy.
  - score matmuls read the upper head via tile_position=(64, 0); score
    psum tiles group SAME-PARITY heads so a single psum bank never mixes
    tile positions (mixing faults the device).
  - band/validity mask multiply on DVE in fp16 SBUF (2x perf mode).
  - attnT built with DMA transposes (14ns/16x128-tile) instead of PE
    transposes + DVE copies.
  - wqk host layout is pair-major so per-pair weight DMAs are contiguous;
    DMA queues: SP=wqk then out, scalar=x, gpsimd=wv/mask/wp/bias.
"""

import numpy as np

import concourse.bass as bass
import concourse.tile as tile
from concourse import bacc, mybir
from concourse._compat import with_exitstack

B, N, C = 2, 2048, 768
H, D = 12, 64
HALF = 64            # half window
NCORES = 8
CHUNK = 512          # queries per core
NK = CHUNK + 2 * HALF  # 640 rows incl halo
SCALE = D ** -0.5

F16 = mybir.dt.float16
F32 = mybir.dt.float32


@with_exitstack
def attn_core_kernel(ctx, tc, outs, ins, repeat=1):
    nc = tc.nc
    out_ap = outs["out"]
    xT, wqkT, wvT, wpT, bias, maskT = (
        ins["xT"], ins["wqkT"], ins["wvT"], ins["wpT"], ins["bias"], ins["maskT"],
    )

    consts = ctx.enter_context(tc.tile_pool(name="consts", bufs=1))
    ppool = ctx.enter_context(tc.tile_pool(name="ps", bufs=3, space="PSUM"))
    scpool = ctx.enter_context(tc.tile_pool(name="scp", bufs=3, space="PSUM"))
    avpool = ctx.enter_context(tc.tile_pool(name="avp", bufs=2, space="PSUM"))
    ptpool = ctx.enter_context(tc.tile_pool(name="pt", bufs=18))
    rcpool = ctx.enter_context(tc.tile_pool(name="rc", bufs=4))
    aqpool = ctx.enter_context(tc.tile_pool(name="aq", bufs=3))
    outpool = ctx.enter_context(tc.tile_pool(name="ob", bufs=2))

    xT_sb = consts.tile([128, 6, NK], F16)
    wqk_sb = consts.tile([128, 12, 768], F16)   # pair-major: [p, pair, ct*128]
    wv_sb = consts.tile([128, 6, 768], F16)
    wp_sb = consts.tile([128, 6, 768], F16)
    mask_sb = consts.tile([128, 5, 256], F16)
    bias_sb = consts.tile([128, 768], F32)
    qkq_sb = consts.tile([128, 6, CHUNK], F16)  # q pairs [q-local]
    qkk_sb = consts.tile([128, 6, NK], F16)     # k pairs [key incl halo]
    vaug_sb = consts.tile([128, 5, H * 65], F16)  # [key-tile, head*(64+ones)]
    attnT_sb = consts.tile([128, 6, CHUNK], F16)  # [c-tile, q]
    ones_set = [False]

    # ---- loads ----
    xT3 = xT.rearrange("(t p) n -> p t n", p=128)
    wqk3 = wqkT.rearrange("p (g e) -> p g e", e=768)   # host is already [128, 12, 768]
    wv3 = wvT.rearrange("(t p) e -> p t e", p=128)
    wp3 = wpT.rearrange("(t p) e -> p t e", p=128)

    def loads():
        # SP: wqk pair-by-pair (contiguous per pair; pair 0 first so PE can
        # start), then wv/wp (first used much later)
        for g in range(12):
            nc.sync.dma_start(wqk_sb[:, g, :], wqk3[:, g, :])
        for t in range(6):
            nc.sync.dma_start(wv_sb[:, t, :], wv3[:, t, :])
        for t in range(6):
            nc.sync.dma_start(wp_sb[:, t, :], wp3[:, t, :])
        # x tiles split across the two other queues for parallel arrival
        for t in range(3):
            nc.scalar.dma_start(xT_sb[:, t, :], xT3[:, t, :])
        for t in range(3, 6):
            nc.gpsimd.dma_start(xT_sb[:, t, :], xT3[:, t, :])
        for kt in range(5):
            nc.gpsimd.dma_start(mask_sb[:, kt, :], maskT[kt])
        nc.gpsimd.dma_start(bias_sb[:], bias[0:1, :].to_broadcast((128, 768)))
        if not ones_set[0]:
            ones_set[0] = True
            va = vaug_sb.rearrange("p t (h u) -> p t h u", u=65)
            for kt in range(5):
                nc.vector.memset(va[:, kt, :, 64], 1.0)

    evac_flip = [0]

    def qk_phase(phase):
        # qkT: two 64-row head-groups per M=128 matmul; single full-height
        # evacuation per chunk, alternating DVE / ACT.
        # phase A: q cols 64-448 (q-local 0-384), k cols 0-384 -> enough for
        # scores_kt 0..2; phase B: the rest.
        for g in range(12):        # 0-5: q pairs, 6-11: k pairs
            if phase == 0:
                c0, w = (64, 384) if g < 6 else (0, 384)
            else:
                c0, w = (448, 128) if g < 6 else (384, 256)
            ps = ppool.tile([128, 512], F32, tag="mm")
            for ct in range(6):
                nc.tensor.matmul(
                    ps[:, :w],
                    wqk_sb[:, g, ct * 128:(ct + 1) * 128],
                    xT_sb[:, ct, c0:c0 + w],
                    start=(ct == 0), stop=(ct == 5),
                )
            dst = (qkq_sb[:, g, c0 - 64:c0 - 64 + w] if g < 6
                   else qkk_sb[:, g - 6, c0:c0 + w])
            # phase A alternates DVE/ACT (ACT is idle before exp starts);
            # phase B runs while ACT does exp, so keep it off ACT
            if phase == 1 or evac_flip[0] % 2 == 0:
                nc.vector.tensor_copy(out=dst, in_=ps[:, :w])
            else:
                nc.scalar.copy(out=dst, in_=ps[:, :w])
            evac_flip[0] += 1

    def v_nt(nt):
        # v -> vaug (strided per-head destination)
        va = vaug_sb.rearrange("p t (h u) -> p t h u", u=65)
        for c0, w, h0, nh in ((0, 512, 0, 8), (512, 256, 8, 4)):
            ps = ppool.tile([128, 512], F32, tag="mm")
            for ct in range(6):
                nc.tensor.matmul(
                    ps[:, :w],
                    xT_sb[:, ct, nt * 128:(nt + 1) * 128],
                    wv_sb[:, ct, c0:c0 + w],
                    start=(ct == 0), stop=(ct == 5),
                )
            nc.vector.tensor_copy(
                out=va[:, nt, h0:h0 + nh, 0:64],
                in_=ps[:, :w].rearrange("p (h d) -> p h d", d=64),
            )

    pt_tiles = {}

    def scores_kt(kt):
        # query range actually consumed downstream (q-local coords for the
        # window [kt*128-128, kt*128+128))
        cq0, cq1 = (128, 256) if kt == 0 else ((0, 128) if kt == 4 else (0, 256))
        # score tiles group SAME-PARITY heads: (4i+par, 4i+2+par) so one psum
        # bank only sees one tile_position
        for i in range(3):
            for par in range(2):
                rows = slice(64 * par, 64 * par + 64)
                sc = scpool.tile([128, 512], F32, tag="sc")
                for j in range(2):
                    h = 4 * i + 2 * j + par
                    lhsT = qkk_sb[rows, h // 2, kt * 128:kt * 128 + 128]
                    rhs = qkq_sb[rows, h // 2,
                                 kt * 128 - 128 + cq0:kt * 128 - 128 + cq1]
                    nc.tensor.matmul(sc[:, 256 * j + cq0:256 * j + cq1],
                                     lhsT, rhs, start=True, stop=True,
                                     tile_position=(64 * par, 0))
                pt = ptpool.tile([128, 512], F16, tag="pt")
                sc2 = sc.rearrange("p (h q) -> p h q", h=2)
                pt2 = pt.rearrange("p (h q) -> p h q", h=2)
                nc.scalar.activation(out=pt2[:, :, cq0:cq1], in_=sc2[:, :, cq0:cq1],
                                     func=mybir.ActivationFunctionType.Exp)
                nc.gpsimd.tensor_tensor(
                    pt2[:, :, cq0:cq1], pt2[:, :, cq0:cq1],
                    mask_sb[:, kt:kt + 1, cq0:cq1].to_broadcast((128, 2, cq1 - cq0)),
                    mybir.AluOpType.mult,
                )
                pt_tiles[(kt, i, par)] = pt

    def av_r(r):
        va = vaug_sb.rearrange("p t (h u) -> p t h u", u=65)
        aq = aqpool.tile([128, 768], F16, tag="aq")
        for hg in range(3):
            av = avpool.tile([128, 260], F32, tag="av")
            av3 = av.rearrange("p (h u) -> p h u", u=65)
            for j in range(4):
                h = 4 * hg + j
                i, par, blk = h // 4, h & 1, (h // 2) & 1
                for ki, kt in ((0, r), (1, r + 1)):
                    col0 = 128 if ki == 0 else 0
                    pt = pt_tiles[(kt, i, par)]
                    lhsT = pt[:, 256 * blk + col0:256 * blk + col0 + 128]
                    nc.tensor.matmul(av3[:, j, :], lhsT, va[:, kt, h, :],
                                     start=(ki == 0), stop=(ki == 1))
            rc = rcpool.tile([128, 4], F32, tag="rc")
            nc.vector.reciprocal(rc[:], av3[:, :, 64])
            aq4 = aq.rearrange("p (h d) -> p h d", d=64)
            if r < 3:
                nc.vector.tensor_tensor(
                    aq4[:, 4 * hg:4 * hg + 4, :],
                    av3[:, :, 0:64],
                    rc[:, :, None].to_broadcast((128, 4, 64)),
                    mybir.AluOpType.mult,
                )
            else:
                # last round: ACT (done with exp) evacuates av and Pool (done
                # with masks) normalizes, freeing DVE for the tail bias-adds
                avs = rcpool.tile([128, 4, 64], F32, tag="avs")
                nc.scalar.copy(out=avs[:], in_=av3[:, :, 0:64])
                nc.gpsimd.tensor_tensor(
                    aq4[:, 4 * hg:4 * hg + 4, :],
                    avs[:],
                    rc[:, :, None].to_broadcast((128, 4, 64)),
                    mybir.AluOpType.mult,
                )
        # transpose [q, c] -> attnT [c, q] via DMA transposes (one per c-tile)
        qsl = slice(128 * r, 128 * r + 128)
        for ct in range(6):
            nc.sync.dma_start(attnT_sb[:, ct, qsl],
                              aq[:, ct * 128:(ct + 1) * 128], transpose=True)

    def proj_r(r):
        ob = outpool.tile([128, 768], F32, tag="ob")
        # the final round tapers to 128-col chunks so the last bias-add and
        # out-DMA exposure after the last matmul is minimal
        chunks = ((0, 256), (256, 256), (512, 256)) if r < 3 else \
                 ((0, 256), (256, 256), (512, 128), (640, 128))
        for ci, (c0, w) in enumerate(chunks):
            ps = ppool.tile([128, 512], F32, tag="mm")
            for ct in range(6):
                nc.tensor.matmul(
                    ps[:, :w],
                    attnT_sb[:, ct, 128 * r:128 * r + 128],
                    wp_sb[:, ct, c0:c0 + w],
                    start=(ct == 0), stop=(ct == 5),
                )
            nc.vector.tensor_add(out=ob[:, c0:c0 + w], in0=ps[:, :w],
                                 in1=bias_sb[:, c0:c0 + w])
            # stream each column chunk out as soon as its bias-add lands;
            # the very last chunk goes on the otherwise-idle gpsimd queue
            q = nc.gpsimd if (r == 3 and ci == 3) else nc.sync
            q.dma_start(out_ap[128 * r:128 * r + 128, c0:c0 + w],
                        ob[:, c0:c0 + w])

    for _rep in range(repeat):
        pt_tiles.clear()
        loads()
        # software pipeline: phase-A qk unlocks scores for kt 0-2 early so
        # exp/mask (ACT/Pool) overlap the remaining projection matmuls
        qk_phase(0)
        scores_kt(0)
        scores_kt(1)
        scores_kt(2)
        v_nt(0)
        v_nt(1)
        av_r(0)
        qk_phase(1)
        v_nt(2)
        scores_kt(3)
        av_r(1)
        proj_r(0)
        v_nt(3)
        v_nt(4)
        scores_kt(4)
        av_r(2)
        av_r(3)
        proj_r(1)
        proj_r(2)
        proj_r(3)


def build_nc(repeat=1):
    nc = bacc.Bacc("TRN2", target_bir_lowering=False, debug=False)
    ins = {
        "xT": nc.dram_tensor("xT", [C, NK], F16, kind="ExternalInput").ap(),
        "wqkT": nc.dram_tensor("wqkT", [128, 12 * 768], F16, kind="ExternalInput").ap(),
        "wvT": nc.dram_tensor("wvT", [C, C], F16, kind="ExternalInput").ap(),
        "wpT": nc.dram_tensor("wpT", [C, C], F16, kind="ExternalInput").ap(),
        "bias": nc.dram_tensor("bias", [1, C], F32, kind="ExternalInput").ap(),
        "maskT": nc.dram_tensor("maskT", [5, 128, 256], F16, kind="ExternalInput").ap(),
    }
    outs = {"out": nc.dram_tensor("out", [CHUNK, C], F32, kind="ExternalOutput").ap()}
    with tile.TileContext(nc) as tc:
        attn_core_kernel(tc, outs, ins, repeat=repeat)
    nc.finalize()
    return nc


def make_core_inputs(x, w_qkv, w_proj, b_proj):
    """Build the 8 per-core input maps from full inputs."""
    x = np.asarray(x, dtype=np.float32)
    w_qkv = np.asarray(w_qkv, dtype=np.float32)
    w_proj = np.asarray(w_proj, dtype=np.float32)
    b_proj = np.asarray(b_proj, dtype=np.float32)

    # wqk pair-major: [p=128, pair g, ct, 128] where the matmul lhsT for
    # (g, ct) is wqk[ct*128 + p, g*128 + col]  (wqkT rows = contraction c,
    # cols = qk output channel; q channels 0..768 scaled, k next)
    wqk = np.concatenate([w_qkv[:C] * SCALE, w_qkv[C:2 * C]], axis=0)  # [2C, C]
    wqkT = np.ascontiguousarray(wqk.T).astype(np.float16)  # [c, e] e in [0,1536)
    wqk4 = wqkT.reshape(6, 128, 12, 128).transpose(1, 2, 0, 3)  # [p, pair, ct, 128]
    wqk_host = np.ascontiguousarray(wqk4.reshape(128, 12 * 768))

    wvT = np.ascontiguousarray(w_qkv[2 * C:].T).astype(np.float16)
    wpT = np.ascontiguousarray(w_proj.T).astype(np.float16)
    bias = b_proj.reshape(1, C).astype(np.float32)

    in_maps = []
    for c in range(NCORES):
        b, s = divmod(c, 4)
        lo = s * CHUNK - HALF
        hi = s * CHUNK + CHUNK + HALF
        xs = np.zeros((NK, C), dtype=np.float32)
        s0, s1 = max(lo, 0), min(hi, N)
        xs[s0 - lo:s1 - lo] = x[b, s0:s1]
        xT = np.ascontiguousarray(xs.T).astype(np.float16)

        mask = np.zeros((5, 128, 256), dtype=np.float16)
        k = np.arange(128)[:, None]
        cq = np.arange(256)[None, :]
        band = (cq - k >= 0) & (cq - k <= 128)
        for kt in range(5):
            key_seq = s * CHUNK - HALF + 128 * kt + k
            valid = (key_seq >= 0) & (key_seq < N)
            mask[kt] = (band & valid).astype(np.float16)

        in_maps.append({
            "xT": xT, "wqkT": wqk_host, "wvT": wvT, "wpT": wpT,
            "bias": bias, "maskT": mask,
        })
    return in_maps


_NC_CACHE = None


def kernel(x, w_qkv, w_proj, b_proj):
    from concourse.bass_utils import run_bass_kernel_spmd

    global _NC_CACHE
    if _NC_CACHE is None:
        _NC_CACHE = build_nc()
    in_maps = make_core_inputs(x, w_qkv, w_proj, b_proj)
    res = run_bass_kernel_spmd(_NC_CACHE, in_maps, core_ids=list(range(NCORES)))
    out = np.empty((B, N, C), dtype=np.float32)
    for c in range(NCORES):
        b, s = divmod(c, 4)
        out[b, s * CHUNK:(s + 1) * CHUNK] = res.results[c]["out"]
    return out
